# revision 1
# baseline (speedup 1.0000x reference)
"""Sharded kNN (retrieval) kernel for 8 Trainium2 NeuronCores.

Strategy (per the classic sharded-kNN reduction):
  - Shard X_train / Y_train along N across 8 cores (12500 rows each).
  - Each core computes scores s[b, n] = x_b . t_n - |t_n|^2/2 for its shard
    (argmax of s  <=>  argmin of euclidean distance), via fp16 matmuls on the
    tensor engine (queries stationary, X^T streamed), then finds the top-8
    candidates per query with the DVE MAX8/FIND_INDEX8 instructions, and
    re-ranks those 8 exactly in fp32 (gather rows by indirect DMA +
    tensor_tensor_reduce dot products).
  - Each core outputs (exact best score, local argmin) per query; the host
    does the tiny 8-way (min, argmin) reduction and gathers Y_train rows.

The fp16 first pass has score noise ~0.02 while the top-1/top-2 score gap is
~7 (d^2 gap ~15), so the true argmin is in the fp16 top-8 with overwhelming
probability; the fp32 re-rank then reproduces the reference's exact ordering.
"""

import os
import numpy as np
from contextlib import ExitStack

import concourse.bass as bass
import concourse.mybir as mybir
import concourse.tile as tile
from concourse.bass_utils import run_bass_kernel_spmd

# Problem shape (hardcoded per contest contract).
N_CORES = 8
B = 1024          # queries
D = 768           # feature dim (48*16)
N = 100000        # training rows
N_LOC = N // N_CORES          # 12500 rows per core
P = 128                       # partitions
BT = B // P                   # 8 query tiles
NCHUNK = 500                  # candidates per PSUM chunk
NCHUNKS = N_LOC // NCHUNK     # 25
KC = 128                      # contraction tile
KCHUNKS = D // KC             # 6
TOPK = 8
PAD = 776                     # 768 + 1 (t2/2) + 7 zero pad -> 3104B rows (32B aligned)
# Tie-break sentinel: must stay exactly representable in fp32 when combined
# with indices < N_LOC (so idx - BIG is exact), i.e. well under 2^24.
BIG = 1.0e6

_F16 = mybir.dt.float16
_F32 = mybir.dt.float32
_U32 = mybir.dt.uint32


def _split_sync_waits(nc, maxw=1):
    """Workaround for this walrus build: it accepts at most ONE sync-wait
    command per instruction.  Move extra sem waits onto preceding same-engine
    nops (same queue => executed in order before the instruction)."""
    from bass_rust import InstNoOp

    n_split = 0
    for f in nc.m.functions:
        for blk in f.blocks:
            insts = blk.instructions
            i = 0
            while i < len(insts):
                inst = insts[i]
                si = inst.sync_info
                ow = list(si.on_wait) if (si is not None and si.on_wait) else []
                if len(ow) > maxw:
                    keep, extra = ow[-maxw:], ow[:-maxw]
                    inst.sync_info = mybir.SyncInfo(
                        on_wait=keep, on_update=list(si.on_update or [])
                    )
                    nops = []
                    for j in range(0, len(extra), maxw):
                        nop = InstNoOp(name=f"{inst.name}-ws{j}", ins=[], outs=[])
                        nop.engine = inst.engine
                        nop.sync_info = mybir.SyncInfo(
                            on_wait=extra[j : j + maxw], on_update=[]
                        )
                        nops.append(nop)
                    insts[i:i] = nops
                    i += len(nops)
                    n_split += 1
                i += 1
    return n_split


def _build():
    nc = bass.Bass()
    xt16 = nc.dram_tensor("xt16", [D, B], _F16, kind="ExternalInput")
    xe32 = nc.dram_tensor("xe32", [B, PAD], _F32, kind="ExternalInput")
    xtr16 = nc.dram_tensor("xtr16", [D, N_LOC], _F16, kind="ExternalInput")
    v16 = nc.dram_tensor("v16", [1, N_LOC], _F16, kind="ExternalInput")
    xg32 = nc.dram_tensor("xg32", [N_LOC, PAD], _F32, kind="ExternalInput")
    out_val = nc.dram_tensor("out_val", [B, 1], _F32, kind="ExternalOutput")
    out_idx = nc.dram_tensor("out_idx", [B, 1], _F32, kind="ExternalOutput")

    # DRAM views with the k-chunk structure exposed: (k p) n -> p k n
    xt16_v = xt16[:, :].rearrange("(k p) b -> p k b", p=P)
    xtr16_v = xtr16[:, :].rearrange("(k p) n -> p k n", p=P)

    with ExitStack() as ctx:
        tc = ctx.enter_context(tile.TileContext(nc))
        const_pool = ctx.enter_context(tc.tile_pool(name="const", bufs=1))
        xq_pool = ctx.enter_context(tc.tile_pool(name="xq", bufs=2))
        xe_pool = ctx.enter_context(tc.tile_pool(name="xe", bufs=2))
        xtr_pool = ctx.enter_context(tc.tile_pool(name="xtr", bufs=3))
        score_pool = ctx.enter_context(tc.tile_pool(name="scores", bufs=2))
        top_pool = ctx.enter_context(tc.tile_pool(name="top", bufs=2))
        gather_pool = ctx.enter_context(tc.tile_pool(name="gather", bufs=1))
        psum_pool = ctx.enter_context(tc.tile_pool(name="psum", bufs=8, space="PSUM"))

        ones16 = const_pool.tile([1, P], _F16)
        nc.vector.memset(ones16[:], 1.0)
        vrow = const_pool.tile([1, N_LOC], _F16)
        nc.sync.dma_start(vrow[:], v16[:, :])

        for bt in range(BT):
            bs = slice(bt * P, (bt + 1) * P)
            xq = xq_pool.tile([P, KCHUNKS, KC], _F16)
            nc.sync.dma_start(xq[:], xt16_v[:, :, bs])
            xe = xe_pool.tile([P, PAD], _F32)
            nc.sync.dma_start(xe[:], xe32[bs, :])

            scores = score_pool.tile([P, N_LOC], _F32)
            for c in range(NCHUNKS):
                ns = slice(c * NCHUNK, (c + 1) * NCHUNK)
                xtr = xtr_pool.tile([P, KCHUNKS, NCHUNK], _F16)
                nc.sync.dma_start(xtr[:], xtr16_v[:, :, ns])
                ps = psum_pool.tile([P, NCHUNK], _F32)
                for kc in range(KCHUNKS):
                    nc.tensor.matmul(
                        ps[:],
                        lhsT=xq[:, kc, :],
                        rhs=xtr[:, kc, :],
                        start=(kc == 0),
                        stop=False,
                    )
                # += v (the -|t|^2/2 bias) via a K=1 matmul of ones^T @ v_chunk
                nc.tensor.matmul(
                    ps[:], lhsT=ones16[:], rhs=vrow[:, ns], start=False, stop=True
                )
                nc.scalar.copy(scores[:, ns], ps[:])

            # top-8 approximate candidates per query
            tv = top_pool.tile([P, TOPK], _F32)
            ti = top_pool.tile([P, TOPK], _U32)
            nc.vector.max(tv[:], scores[:])
            nc.vector.max_index(ti[:], tv[:], scores[:])

            # gather the 8 candidate rows (768 feats + t2/2 + pad) per query
            xg = gather_pool.tile([P, TOPK, PAD], _F32)
            for j in range(TOPK):
                nc.gpsimd.indirect_dma_start(
                    out=xg[:, j, :],
                    out_offset=None,
                    in_=xg32[:, :],
                    in_offset=bass.IndirectOffsetOnAxis(ap=ti[:, j : j + 1], axis=0),
                )

            # exact fp32 re-rank: cand[j] = xe . xg[j] = x.t - t2/2
            cand = top_pool.tile([P, TOPK], _F32)
            scratch = gather_pool.tile([P, PAD], _F32)
            for j in range(TOPK):
                nc.vector.scalar_tensor_tensor(
                    out=scratch[:],
                    in0=xg[:, j, :],
                    scalar=0.0,
                    in1=xe[:],
                    op0=mybir.AluOpType.add,
                    op1=mybir.AluOpType.mult,
                    accum_out=cand[:, j : j + 1],
                )

            bv = top_pool.tile([P, 1], _F32)
            nc.vector.tensor_reduce(
                bv[:], cand[:], axis=mybir.AxisListType.X, op=mybir.AluOpType.max
            )
            # pick the smallest original index among exact-score ties
            tif = top_pool.tile([P, TOPK], _F32)
            nc.vector.tensor_copy(tif[:], ti[:])
            eq = top_pool.tile([P, TOPK], _F32)
            nc.vector.tensor_scalar(
                eq[:], cand[:], bv[:], None, op0=mybir.AluOpType.is_equal
            )
            t1 = top_pool.tile([P, TOPK], _F32)
            nc.vector.scalar_tensor_tensor(
                t1[:],
                in0=tif[:],
                scalar=BIG,
                in1=eq[:],
                op0=mybir.AluOpType.subtract,
                op1=mybir.AluOpType.mult,
            )
            masked = top_pool.tile([P, TOPK], _F32)
            nc.vector.tensor_scalar_add(masked[:], t1[:], BIG)
            bi = top_pool.tile([P, 1], _F32)
            nc.vector.tensor_reduce(
                bi[:], masked[:], axis=mybir.AxisListType.X, op=mybir.AluOpType.min
            )

            nc.sync.dma_start(out_val[bs, :], bv[:])
            nc.sync.dma_start(out_idx[bs, :], bi[:])

    _split_sync_waits(nc)
    return nc


_NC_CACHE = None
LAST_RESULTS = None  # BassKernelResults of the most recent run (for test harness)


def prepare_in_maps(x, X_train):
    x = np.asarray(x, dtype=np.float32)
    X_train = np.asarray(X_train, dtype=np.float32)

    x_flat = np.ascontiguousarray(x.reshape(B, D))
    xt16 = np.ascontiguousarray(x_flat.T).astype(np.float16)
    xe32 = np.concatenate(
        [x_flat, -np.ones((B, 1), np.float32), np.zeros((B, PAD - D - 1), np.float32)],
        axis=1,
    )
    xe32 = np.ascontiguousarray(xe32)

    in_maps = []
    for c in range(N_CORES):
        Xc = X_train[c * N_LOC : (c + 1) * N_LOC]
        t2 = (Xc.astype(np.float64) ** 2).sum(axis=1)
        xtr16 = np.ascontiguousarray(Xc.T).astype(np.float16)
        v16 = ((t2.mean() - t2) * 0.5).astype(np.float16)[None, :]
        xg32 = np.concatenate(
            [
                Xc,
                (t2 * 0.5).astype(np.float32)[:, None],
                np.zeros((N_LOC, PAD - D - 1), np.float32),
            ],
            axis=1,
        )
        in_maps.append(
            {
                "xt16": xt16,
                "xe32": xe32,
                "xtr16": np.ascontiguousarray(xtr16),
                "v16": np.ascontiguousarray(v16),
                "xg32": np.ascontiguousarray(xg32),
            }
        )
    return in_maps


def kernel(x, X_train, Y_train):
    global _NC_CACHE, LAST_RESULTS
    Y_train = np.asarray(Y_train)
    in_maps = prepare_in_maps(x, X_train)

    if _NC_CACHE is None:
        _NC_CACHE = _build()

    LAST_RESULTS = run_bass_kernel_spmd(
        _NC_CACHE,
        in_maps,
        core_ids=list(range(N_CORES)),
    )
    results = LAST_RESULTS.results

    vals = np.stack([r["out_val"][:, 0] for r in results])  # [8, B]
    idxs = np.stack([r["out_idx"][:, 0] for r in results])  # [8, B]
    win = np.argmax(vals, axis=0)  # first core on ties == smallest global index
    nearest = idxs[win, np.arange(B)].astype(np.int64) + win * N_LOC
    return Y_train[nearest]



# revision 11
# speedup vs baseline: 46.6295x; 46.6295x over previous
"""Sharded kNN (retrieval) kernel for 8 Trainium2 NeuronCores.

Strategy (classic sharded-kNN reduction):
  - Shard X_train / Y_train along N across 8 cores (12500 rows each, padded
    to 13312 = 13 chunks x 1024).
  - Each core holds its whole X shard SBUF-resident in fp8e4 (~10 MB loaded
    once, as 13 pipelined chunk DMAs so the first matmuls start ~5 us in,
    instead of re-streaming 19+ MB of fp16 from HBM for every query tile).
  - Scores s[b, n] = x_b . t_n - |t_n|^2/2 (argmax of s <=> argmin of
    euclidean distance) via fp8 matmuls, 1024-wide PSUM chunks (2 banks),
    bias accumulated with a K=1 fp8 matmul in the same PSUM group.
  - Top-8 per query: the fp32 score row is scanned in 3 segments with DVE
    MAX8 + FIND_INDEX8 as soon as each segment is ready (overlapping the
    tensor engine); the 3x8 (value, index) candidates are merged with an
    8-wide MAX8 + FIND_INDEX8 + one-hot dot-product index translation, all
    on 24-element arrays, so no full-row scan sits on the critical path.
  - Exact fp32 re-rank of the 8 merged candidates (indirect-DMA row gather +
    DVE dot products) reproduces the reference's fp32 ordering.
  - Each core outputs (exact best score, local argmin) per query; the host
    does the tiny 8-way (min, argmin) reduction and gathers Y_train rows.

The fp8 first pass has score noise ~1.3 while the top-1/top-2 score gap is
typically ~5 (d^2 gap ~10), so the true argmin lands in the fp8 top-8 with
overwhelming probability (measured: worst rank 2 over all 1024 queries);
the fp32 re-rank then reproduces the reference's exact ordering.
"""

import numpy as np
from contextlib import ExitStack

import concourse.bass as bass
import concourse.mybir as mybir
import concourse.tile as tile
from concourse.bass_utils import run_bass_kernel_spmd

# Problem shape (hardcoded per contest contract).
N_CORES = 8
B = 1024          # queries
D = 768           # feature dim (48*16)
N = 100000        # training rows
N_LOC = N // N_CORES          # 12500 rows per core
P = 128                       # partitions
BT = B // P                   # 8 query tiles
NCHUNK = 512                  # candidates per PSUM chunk (one bank)
NCHUNKS = 26                  # 26 x 512 = 13312 (padded)
N_PAD = NCHUNKS * NCHUNK      # 13312
# chunk ranges scanned as independent segments (MAX8 + FIND_INDEX8 each)
SEGS = [(0, 10), (10, 20), (20, 26)]
NSEG = len(SEGS)
KC = 128                      # contraction tile
KCHUNKS = D // KC             # 6
TOPK = 8
NCAND = NSEG * TOPK           # 24 merged candidates
PAD = 776                     # 768 + 1 (t2/2) + 7 zero pad -> 3104B rows (32B aligned)
# Tie-break sentinel: must stay exactly representable in fp32 when combined
# with indices < N_PAD (so idx - BIG is exact), i.e. well under 2^24.
BIG = 1.0e6
PAD_SCORE = -240.0            # pass-1 score of padded candidates (fp8e4 min finite;
#                               real top-8 scores are ~+40..+130, so pads never rank)

_F8 = mybir.dt.float8e4
_F32 = mybir.dt.float32
_I32 = mybir.dt.int32
_U32 = mybir.dt.uint32


def _split_sync_waits(nc, maxw=1):
    """Workaround for this walrus build: it accepts at most ONE sync-wait
    command per instruction.  Move extra sem waits onto preceding same-engine
    nops (same queue => executed in order before the instruction)."""
    from bass_rust import InstNoOp

    n_split = 0
    for f in nc.m.functions:
        for blk in f.blocks:
            insts = blk.instructions
            i = 0
            while i < len(insts):
                inst = insts[i]
                si = inst.sync_info
                ow = list(si.on_wait) if (si is not None and si.on_wait) else []
                if len(ow) > maxw:
                    keep, extra = ow[-maxw:], ow[:-maxw]
                    inst.sync_info = mybir.SyncInfo(
                        on_wait=keep, on_update=list(si.on_update or [])
                    )
                    nops = []
                    for j in range(0, len(extra), maxw):
                        nop = InstNoOp(name=f"{inst.name}-ws{j}", ins=[], outs=[])
                        nop.engine = inst.engine
                        nop.sync_info = mybir.SyncInfo(
                            on_wait=extra[j : j + maxw], on_update=[]
                        )
                        nops.append(nop)
                    insts[i:i] = nops
                    i += len(nops)
                    n_split += 1
                i += 1
    return n_split


def _build(split_sync_waits=True):
    nc = bass.Bass()
    # Host-side layouts are pre-transposed so every load is contiguous
    # per partition:
    #   xq8  [p, kc, b]    = x_flat[b, kc*128+p]          (fp8e4)
    #   xtr8 [p, c, kc, n] = X_pad[c*1024+n, kc*128+p]    (fp8e4)
    xq8 = nc.dram_tensor("xq8", [P, KCHUNKS * B], _F8, kind="ExternalInput")
    xe32 = nc.dram_tensor("xe32", [B, PAD], _F32, kind="ExternalInput")
    xtr8 = nc.dram_tensor(
        "xtr8", [P, NCHUNKS * KCHUNKS * NCHUNK], _F8, kind="ExternalInput"
    )
    v8 = nc.dram_tensor("v8", [1, N_PAD], _F8, kind="ExternalInput")
    xg32 = nc.dram_tensor("xg32", [N_PAD, PAD], _F32, kind="ExternalInput")
    out_val = nc.dram_tensor("out_val", [B, 1], _F32, kind="ExternalOutput")
    out_idx = nc.dram_tensor("out_idx", [B, 1], _F32, kind="ExternalOutput")

    with ExitStack() as ctx:
        tc = ctx.enter_context(tile.TileContext(nc))
        const_pool = ctx.enter_context(tc.tile_pool(name="const", bufs=1))
        xe_pool = ctx.enter_context(tc.tile_pool(name="xe", bufs=2))
        score_pool = ctx.enter_context(tc.tile_pool(name="scores", bufs=1))
        top_pool = ctx.enter_context(tc.tile_pool(name="top", bufs=2))
        gather_pool = ctx.enter_context(tc.tile_pool(name="gather", bufs=1))
        psum_pool = ctx.enter_context(tc.tile_pool(name="psum", bufs=8, space="PSUM"))

        ones8 = const_pool.tile([1, P], _F8)
        nc.vector.memset(ones8[:], 1.0)
        iota_i = const_pool.tile([P, NCAND], _I32)
        nc.gpsimd.iota(iota_i[:], [[1, NCAND]], channel_multiplier=0)
        iota_f = const_pool.tile([P, NCAND], _F32)
        nc.vector.tensor_copy(iota_f[:], iota_i[:])
        vrow = const_pool.tile([1, N_PAD], _F8)
        nc.sync.dma_start(vrow[:], v8[:, :])
        # all query tiles, fp8-stationary operands: [128, 6, 1024] = 6.1 KB
        xq = const_pool.tile([P, KCHUNKS, B], _F8)
        nc.sync.dma_start(xq[:], xq8[:, :].rearrange("p (k b) -> p k b", b=B))
        # whole X shard, fp8, resident (78 KB/partition), loaded as per-chunk
        # subview DMAs so chunk-c matmuls start as soon as chunk c lands.
        xtr = const_pool.tile([P, NCHUNKS, KCHUNKS, NCHUNK], _F8)
        xtr8_v = xtr8[:, :].rearrange(
            "p (c k n) -> p c k n", k=KCHUNKS, n=NCHUNK
        )
        for c in range(NCHUNKS):
            nc.sync.dma_start(xtr[:, c], xtr8_v[:, c])

        for bt in range(BT):
            bs = slice(bt * P, (bt + 1) * P)
            xe = xe_pool.tile([P, PAD], _F32)
            nc.sync.dma_start(xe[:], xe32[bs, :])

            scores = score_pool.tile([P, N_PAD], _F32)
            vh = top_pool.tile([P, NCAND], _F32)   # seg s top-8 values
            ihu = top_pool.tile([P, NCAND], _U32)  # seg s top-8 indices (seg-local)
            for c in range(NCHUNKS):
                ns = slice(c * NCHUNK, (c + 1) * NCHUNK)
                ps = psum_pool.tile([P, NCHUNK], _F32)
                for kc in range(KCHUNKS):
                    nc.tensor.matmul(
                        ps[:],
                        lhsT=xq[:, kc, bs],
                        rhs=xtr[:, c, kc, :],
                        start=(kc == 0),
                        stop=False,
                    )
                # += v (the -|t|^2/2 bias) via a K=1 matmul of ones^T @ v_chunk
                nc.tensor.matmul(
                    ps[:], lhsT=ones8[:], rhs=vrow[:, ns], start=False, stop=True
                )
                nc.scalar.copy(scores[:, ns], ps[:])
                for s, (c0, c1) in enumerate(SEGS):
                    if c == c1 - 1:
                        seg = slice(c0 * NCHUNK, c1 * NCHUNK)
                        sl8 = slice(s * TOPK, (s + 1) * TOPK)
                        nc.vector.max(vh[:, sl8], scores[:, seg])
                        nc.vector.max_index(ihu[:, sl8], vh[:, sl8], scores[:, seg])

            # seg-local indices -> global: convert to f32, add segment offsets
            ihf = top_pool.tile([P, NCAND], _F32)
            nc.vector.tensor_copy(ihf[:], ihu[:])
            for s, (c0, c1) in enumerate(SEGS):
                if c0:
                    sl8 = slice(s * TOPK, (s + 1) * TOPK)
                    nc.vector.tensor_scalar_add(ihf[:, sl8], ihf[:, sl8], float(c0 * NCHUNK))

            # merge: global top-8 values over the 24 candidates, their
            # positions in the 24-array, then one-hot dots to pull indices
            tv = top_pool.tile([P, TOPK], _F32)
            pos = top_pool.tile([P, TOPK], _U32)
            nc.vector.max(tv[:], vh[:])
            nc.vector.max_index(pos[:], tv[:], vh[:])
            posf = top_pool.tile([P, TOPK], _F32)
            nc.vector.tensor_copy(posf[:], pos[:])
            tif = top_pool.tile([P, TOPK], _F32)
            eq24 = top_pool.tile([P, NCAND], _F32)
            s24 = top_pool.tile([P, NCAND], _F32)
            for k in range(TOPK):
                nc.vector.tensor_scalar(
                    eq24[:], iota_f[:], posf[:, k : k + 1], None,
                    op0=mybir.AluOpType.is_equal,
                )
                nc.vector.scalar_tensor_tensor(
                    out=s24[:],
                    in0=eq24[:],
                    scalar=0.0,
                    in1=ihf[:],
                    op0=mybir.AluOpType.add,
                    op1=mybir.AluOpType.mult,
                    accum_out=tif[:, k : k + 1],
                )
            ti = top_pool.tile([P, TOPK], _U32)
            nc.vector.tensor_copy(ti[:], tif[:])

            # gather the 8 candidate rows (768 feats + t2/2 + pad) per query
            xg = gather_pool.tile([P, TOPK, PAD], _F32)
            for j in range(TOPK):
                nc.gpsimd.indirect_dma_start(
                    out=xg[:, j, :],
                    out_offset=None,
                    in_=xg32[:, :],
                    in_offset=bass.IndirectOffsetOnAxis(ap=ti[:, j : j + 1], axis=0),
                )

            # exact fp32 re-rank: cand[j] = xe . xg[j] = x.t - t2/2
            cand = top_pool.tile([P, TOPK], _F32)
            scratch = gather_pool.tile([P, PAD], _F32)
            for j in range(TOPK):
                nc.vector.scalar_tensor_tensor(
                    out=scratch[:],
                    in0=xg[:, j, :],
                    scalar=0.0,
                    in1=xe[:],
                    op0=mybir.AluOpType.add,
                    op1=mybir.AluOpType.mult,
                    accum_out=cand[:, j : j + 1],
                )

            bv = top_pool.tile([P, 1], _F32)
            nc.vector.tensor_reduce(
                bv[:], cand[:], axis=mybir.AxisListType.X, op=mybir.AluOpType.max
            )
            # pick the smallest original index among exact-score ties
            eq = top_pool.tile([P, TOPK], _F32)
            nc.vector.tensor_scalar(
                eq[:], cand[:], bv[:], None, op0=mybir.AluOpType.is_equal
            )
            t1 = top_pool.tile([P, TOPK], _F32)
            nc.vector.scalar_tensor_tensor(
                t1[:],
                in0=tif[:],
                scalar=BIG,
                in1=eq[:],
                op0=mybir.AluOpType.subtract,
                op1=mybir.AluOpType.mult,
            )
            masked = top_pool.tile([P, TOPK], _F32)
            nc.vector.tensor_scalar_add(masked[:], t1[:], BIG)
            bi = top_pool.tile([P, 1], _F32)
            nc.vector.tensor_reduce(
                bi[:], masked[:], axis=mybir.AxisListType.X, op=mybir.AluOpType.min
            )

            nc.sync.dma_start(out_val[bs, :], bv[:])
            nc.sync.dma_start(out_idx[bs, :], bi[:])

    if split_sync_waits:
        _split_sync_waits(nc)
    return nc


_NC_CACHE = None
LAST_RESULTS = None  # BassKernelResults of the most recent run (for test harness)


def prepare_in_maps(x, X_train):
    x = np.asarray(x, dtype=np.float32)
    X_train = np.asarray(X_train, dtype=np.float32)
    f8 = mybir.dt.np(_F8)

    x_flat = np.ascontiguousarray(x.reshape(B, D))
    # [p, kc, b] layout so the one-shot SBUF load is contiguous per partition
    xq8 = (
        x_flat.T.reshape(KCHUNKS, P, B).transpose(1, 0, 2).reshape(P, KCHUNKS * B)
    ).astype(f8)
    xe32 = np.concatenate(
        [x_flat, -np.ones((B, 1), np.float32), np.zeros((B, PAD - D - 1), np.float32)],
        axis=1,
    )
    xe32 = np.ascontiguousarray(xe32)

    in_maps = []
    for c in range(N_CORES):
        Xc = X_train[c * N_LOC : (c + 1) * N_LOC]
        t2 = (Xc.astype(np.float64) ** 2).sum(axis=1)
        Xp = np.zeros((N_PAD, D), np.float32)
        Xp[:N_LOC] = Xc
        # [p, c, kc, n] layout, fp8e4
        xtr8 = (
            Xp.T.reshape(KCHUNKS, P, NCHUNKS, NCHUNK)
            .transpose(1, 2, 0, 3)
            .reshape(P, NCHUNKS * KCHUNKS * NCHUNK)
        ).astype(f8)
        v = np.full(N_PAD, PAD_SCORE, np.float64)
        v[:N_LOC] = (t2.mean() - t2) * 0.5
        v8c = v.astype(f8)[None, :]
        xg32 = np.zeros((N_PAD, PAD), np.float32)
        xg32[:N_LOC, :D] = Xc
        xg32[:N_LOC, D] = (t2 * 0.5).astype(np.float32)
        xg32[N_LOC:, D] = 1.0e9  # re-rank score -1e9 if a pad row ever gathered
        in_maps.append(
            {
                "xq8": np.ascontiguousarray(xq8),
                "xe32": xe32,
                "xtr8": np.ascontiguousarray(xtr8),
                "v8": np.ascontiguousarray(v8c),
                "xg32": np.ascontiguousarray(xg32),
            }
        )
    return in_maps


def kernel(x, X_train, Y_train):
    global _NC_CACHE, LAST_RESULTS
    Y_train = np.asarray(Y_train)
    in_maps = prepare_in_maps(x, X_train)

    if _NC_CACHE is None:
        _NC_CACHE = _build()

    LAST_RESULTS = run_bass_kernel_spmd(
        _NC_CACHE,
        in_maps,
        core_ids=list(range(N_CORES)),
    )
    results = LAST_RESULTS.results

    vals = np.stack([r["out_val"][:, 0] for r in results])  # [8, B]
    idxs = np.stack([r["out_idx"][:, 0] for r in results])  # [8, B]
    win = np.argmax(vals, axis=0)  # first core on ties == smallest global index
    nearest = idxs[win, np.arange(B)].astype(np.int64) + win * N_LOC
    return Y_train[nearest]
